# revision 38
# baseline (speedup 1.0000x reference)
"""Trainium2 Bass kernel for nn_ByteSequenceEmbedder.

Packed data-parallel across 8 NeuronCores: the 16 sequences' VALID positions
(sum of pool_lengths ~= 2048 of 3072 per sequence) are packed into 8 balanced
per-core streams (~4102 cols each instead of 2x3072), cut at word boundaries.
Mid-sequence cuts carry a 2-col receptive-field overlap on each side;
sequence boundaries inside a stream get 2 zero gap cols, and the conv1
left-padding semantics are restored by multiplying x1c with a host-provided
0/1 column mask (per-core DRAM data keeps the SPMD program shared).

Per-core dataflow (channels-on-partitions, fp16 activations, even-aligned
column layout so DVE runs in 2x 16-bit mode):
  embed   : host-precomputed (x0 = tok_emb[tokens] + bpe marker) -> DMA
  conv0   : 3 shifted matmuls per (chunk, co-chunk) in PSUM, DVE relu evac
  highway : 2 blocks x 2 layers; h-path fp16 matmuls, g-path (sigmoid gate)
            fp8e4 DoubleRow matmuls on a gpsimd-made x64 fp8 copy of the
            input; ACT relu/sigmoid evac, DVE combine x' = g*(relu(h)-x)+x
  conv1   : 12 matmuls + DVE relu evac + residual add
  pool    : ragged word max-pool as masked shifted max (additive -60000)
  proj    : fp16 matmuls over all positions, interleaved with hw1l1; host
            gathers word-start columns while unsharding
"""
import numpy as np

import concourse.bacc as bacc
import concourse.tile as tile
import concourse.mybir as mybir

BSZ, NW, T = 16, 1024, 3072
BED, WED = 128, 512
BPE_MASK_IDX = 4
N_CORES = 8
OVL = 2               # receptive-field overlap at mid-sequence cuts
HALO = 2              # left halo cols (even so DVE 2x alignment holds)
F16 = mybir.dt.float16
F32 = mybir.dt.float32
F8 = mybir.dt.float8e4
_F16_NP = np.float16
_F8_NP = mybir.dt.np(F8)
NEG = np.float32(-60000.0)
SX = 64.0             # fp8 activation scale
SW = 256.0            # fp8 g-weight scale
SINV = 1.0 / (SX * SW)

_CACHE = {}


# ---------------------------------------------------------------- packing

def _plan_packing(pool_lengths, n_streams=N_CORES):
    pl = np.asarray(pool_lengths, np.int64)
    starts = np.cumsum(pl, axis=1) - pl

    def try_pack(cap):
        streams = [[] for _ in range(n_streams)]
        zcols = [[] for _ in range(n_streams)]
        lens = [0] * n_streams
        prev_type = [None] * n_streams
        core = 0
        for b in range(BSZ):
            w = 0
            cut = False
            while w < NW:
                if pl[b, w] == 0:
                    w += 1
                    continue
                if core >= n_streams:
                    return None
                if lens[core] == 0:
                    gap, zc = 0, []
                elif prev_type[core] == 'seqend':
                    gap, zc = 2, [lens[core] + 1]
                elif cut:
                    gap, zc = 0, []
                else:
                    gap, zc = 1, [lens[core]]
                pos = lens[core] + gap
                left = OVL if cut else 0
                room = cap - pos - left
                if room < pl[b, w]:
                    core += 1
                    continue
                cum = 0
                w0 = w
                while w < NW and cum + pl[b, w] <= room:
                    cum += pl[b, w]
                    w += 1
                body0 = int(starts[b, w0])
                is_cut_r = w < NW
                right = OVL if is_cut_r else 0
                if pos + left + cum + right > cap:
                    while w > w0 and pos + left + cum + OVL > cap:
                        w -= 1
                        cum -= pl[b, w]
                    if w == w0:
                        core += 1
                        continue
                    is_cut_r = True
                    right = OVL
                s_lo = body0 - left
                s_hi = body0 + cum + right
                streams[core].append(dict(
                    b=b, w0=w0, w1=w, s_lo=s_lo, s_hi=s_hi, pos=pos))
                zcols[core].extend(zc)
                lens[core] = pos + (s_hi - s_lo)
                prev_type[core] = 'cutend' if is_cut_r else 'seqend'
                cut = is_cut_r
        return streams, zcols, lens
    lo = int(np.ceil(pl.sum() / n_streams))
    for cap in range(lo, lo + 4096):
        r = try_pack(cap)
        if r is not None:
            return r[0], r[1], cap
    raise RuntimeError("packing failed")


def _chunk_plan(cap):
    need = cap + 1  # right-context col of a full stream must be computed
    nch = max(1, int(np.ceil(need / 512)))
    w = int(np.ceil(need / nch / 8) * 8)
    if w > 512:
        nch += 1
        w = int(np.ceil(need / nch / 8) * 8)
    return nch, w


# ---------------------------------------------------------------- program

def _build_program(nch, w):
    W = nch * w
    WB = W + 8            # HALO left cols + >=4 right halo, even width
    WP8 = ((W + 15) // 16) * 16   # fp8 copy row width (16B-aligned k-tile step)
    nc = bacc.Bacc("TRN2", target_bir_lowering=False, debug=False)

    def dram_in(name, shape, dt):
        return nc.dram_tensor(name, shape, dt, kind="ExternalInput").ap()

    w_c0 = dram_in("w_c0", [128, 3 * WED], F16)          # [ci, k*512+co]
    w_c18 = dram_in("w_c18", [128, 48, 128], F8)         # conv1 fp8 DR pairs
    w_hh = dram_in("w_hh", [128, 4 * 4 * WED], F16)      # h-half: [(bl*4+q)*512+co]
    w_g8 = dram_in("w_g8", [128, 64, 128], F8)           # g-half fp8 DR pairs
    w_pr = dram_in("w_pr", [128, 4 * WED], F16)          # [q*512+co]
    b_c0 = dram_in("b_c0", [128, 4], F32)
    b_c1 = dram_in("b_c1", [128, 4], F32)
    b_hw = dram_in("b_hw", [128, 4 * 8], F32)            # [bl*8 + m]
    b_pr = dram_in("b_pr", [128, 4], F32)
    x0_in = dram_in("x0_in", [128, WB], F16)             # host-embedded stream
    amsk = dram_in("a_msk", [128, 2 * W], F16)           # pooling additive masks
    m1c = dram_in("m1c", [128, W], F8)                   # x1c zero-col mask

    out = nc.dram_tensor("out", [WED, W], F16, kind="ExternalOutput").ap()

    RELU = mybir.ActivationFunctionType.Relu
    SIGM = mybir.ActivationFunctionType.Sigmoid
    IDEN = mybir.ActivationFunctionType.Identity
    MAX = mybir.AluOpType.max
    ADD = mybir.AluOpType.add
    SUB = mybir.AluOpType.subtract
    MUL = mybir.AluOpType.mult
    DR = mybir.MatmulPerfMode.DoubleRow

    with tile.TileContext(nc) as tc:
        with tc.tile_pool(name="wp", bufs=1) as wp, \
             tc.tile_pool(name="ap", bufs=1) as apool, \
             tc.tile_pool(name="tp", bufs=3) as tp, \
             tc.tile_pool(name="pp", bufs=8, space="PSUM") as pp:

            # ---- HAM warm-up: PE activity from t~0 ----
            wu = wp.tile([128, w], F16)
            nc.vector.memset(wu[:], 0)
            for _ in range(10):
                wps = pp.tile([128, w], F32, tag="ps", name="wps")
                nc.tensor.matmul(out=wps[:], lhsT=wu[:, 0:128], rhs=wu[:],
                                 start=True, stop=True)

            # ---- loads, all on the sync queue (the scalar queue stalls
            # behind the ACT table load). Big tensors are split into several
            # dma_starts so the transfers parallelize across DMA rings;
            # issued in consumption order. ----
            t_x0 = wp.tile([128, WB], F16)
            t_wc0 = wp.tile([128, 3 * WED], F16)
            t_bc0 = wp.tile([128, 4], F32)
            t_whh = wp.tile([128, 4 * 4 * WED], F16)
            t_wg8 = wp.tile([128, 64, 128], F8)
            t_bhw = wp.tile([128, 4 * 8], F32)
            t_wc18 = wp.tile([128, 48, 128], F8)
            t_bc1 = wp.tile([128, 4], F32)
            t_wpr = wp.tile([128, 4 * WED], F16)
            t_bpr = wp.tile([128, 4], F32)

            def dma_split(dst, src, width, pieces):
                step = (width + pieces - 1) // pieces
                for a in range(0, width, step):
                    b = min(width, a + step)
                    nc.sync.dma_start(out=dst[:, a:b], in_=src[:, a:b])

            dma_split(t_wc0, w_c0, 3 * WED, 4)
            dma_split(t_x0, x0_in, WB, 8)
            nc.sync.dma_start(out=t_bc0[:], in_=b_c0[:])
            nc.sync.dma_start(out=t_bhw[:], in_=b_hw[:])
            for bl in range(4):
                for half in range(2):
                    a = bl * 2048 + half * 1024
                    nc.sync.dma_start(out=t_whh[:, a:a + 1024],
                                      in_=w_hh[:, a:a + 1024])
                nc.sync.dma_start(out=t_wg8[:, bl * 16:(bl + 1) * 16, :],
                                  in_=w_g8[:, bl * 16:(bl + 1) * 16, :])
                if bl == 1:
                    nc.sync.dma_start(out=t_wc18[:], in_=w_c18[:])
                    nc.sync.dma_start(out=t_bc1[:], in_=b_c1[:])
            dma_split(t_wpr, w_pr, 4 * WED, 2)
            nc.sync.dma_start(out=t_bpr[:], in_=b_pr[:])
            t_m1 = wp.tile([128, W], F8)
            t_am = wp.tile([128, 2 * W], F16)
            nc.scalar.dma_start(out=t_m1[:], in_=m1c[:])
            dma_split(t_am, amsk, 2 * W, 2)

            def act_buf(tag):
                b = apool.tile([128, 4 * WB], F16, tag=tag, name=tag)
                for q in range(4):
                    nc.vector.memset(b[:, q * WB:q * WB + HALO], 0)
                    nc.vector.memset(b[:, q * WB + HALO + W:(q + 1) * WB], 0)
                return b

            def x8_buf():
                return apool.tile([128, 4, WP8], F8, tag="x8", name="x8", bufs=2)

            scope = nc.named_scope

            def c3(buf, lo, hi, shift=HALO):
                """[128, 4, hi-lo] view of a [128, 4*WB] buffer (c stride WB)."""
                return buf[:].rearrange("p (c x) -> p c x", c=4)[:, :, shift + lo:shift + hi]

            def bc3(row_ap, n_elem):
                """[128, 4, n] broadcast view of a [128, n] row slice."""
                return row_ap.unsqueeze(1).broadcast_to([128, 4, n_elem])

            def x8_copy(src3, x8t, lo, hi):
                nc.vector.tensor_scalar(out=x8t[:, :, lo:hi], in0=src3,
                                        scalar1=SX, scalar2=None, op0=MUL)

            # ---------- conv0 (ACT relu evac) + fp8 copy for hw0l0's gate ----
            with scope("conv0"):
                x1 = act_buf("actA")
                x8a = x8_buf()
                for n in range(nch):
                    lo, hi = n * w, (n + 1) * w
                    for m in range(4):
                        ps = pp.tile([128, w], F32, tag="ps", name="ps")
                        for k in range(3):
                            nc.tensor.matmul(
                                out=ps[:],
                                lhsT=t_wc0[:, k * WED + m * 128:k * WED + (m + 1) * 128],
                                rhs=t_x0[:, n * w + k + 1:n * w + k + 1 + w],
                                start=(k == 0), stop=(k == 2))
                        ys = x1[:, m * WB + HALO + lo:m * WB + HALO + hi]
                        nc.scalar.activation(out=ys, in_=ps[:], func=RELU,
                                             bias=t_bc0[:, m:m + 1], scale=1.0)
                    x8_copy(c3(x1, lo, hi), x8a, lo, hi)

            def highway_layer(X, Y, bl, X8in, X8out, post_chunk=None, fused=True,
                              chunks=None):
                for n in (range(nch) if chunks is None else chunks):
                    lo, hi = n * w, (n + 1) * w
                    hs = []
                    gs = []
                    for m in range(4):
                        ps = pp.tile([128, w], F32, tag="ps", name="ps")
                        for q in range(4):
                            base = (bl * 4 + q) * WED + m * 128
                            nc.tensor.matmul(
                                out=ps[:], lhsT=t_whh[:, base:base + 128],
                                rhs=X[:, q * WB + HALO + lo:q * WB + HALO + hi],
                                start=(q == 0), stop=(q == 3))
                        hs.append(ps)
                    for m in range(4):
                        ps = pp.tile([128, w], F32, tag="ps", name="ps")
                        for p in range(2):
                            idx = ((bl * 4 + m) * 2 + p) * 2
                            nc.tensor.matmul(
                                out=ps[:], lhsT=t_wg8[:, idx:idx + 2, :],
                                rhs=X8in[:, 2 * p:2 * p + 2, lo:hi],
                                start=(p == 0), stop=(p == 1), perf_mode=DR)
                        gs.append(ps)
                    h4 = tp.tile([128, 4 * w], F16, tag="h", name="h4", bufs=2)
                    g4 = tp.tile([128, 4 * w], F16, tag="g", name="g4", bufs=2)
                    for c in range(4):
                        nc.scalar.activation(out=h4[:, c * w:(c + 1) * w],
                                             in_=hs[c][:], func=RELU,
                                             bias=t_bhw[:, bl * 8 + c:bl * 8 + c + 1],
                                             scale=1.0)
                        nc.scalar.activation(out=g4[:, c * w:(c + 1) * w],
                                             in_=gs[c][:], func=SIGM,
                                             bias=t_bhw[:, bl * 8 + 4 + c:bl * 8 + 4 + c + 1],
                                             scale=SINV)
                        if not fused:
                            # per-c combine: shorter latency chain for the
                            # interleaved pool/proj consumers
                            hc = h4[:, c * w:(c + 1) * w]
                            gc = g4[:, c * w:(c + 1) * w]
                            xs = X[:, c * WB + HALO + lo:c * WB + HALO + hi]
                            ys = Y[:, c * WB + HALO + lo:c * WB + HALO + hi]
                            nc.vector.tensor_tensor(out=hc, in0=hc, in1=xs, op=SUB)
                            nc.vector.tensor_tensor(out=hc, in0=hc, in1=gc, op=MUL)
                            nc.vector.tensor_tensor(out=ys, in0=hc, in1=xs, op=ADD)
                    if fused:
                        X3 = c3(X, lo, hi)
                        Y3 = c3(Y, lo, hi)
                        nc.vector.tensor_tensor(out=h4[:], in0=h4[:], in1=X3, op=SUB)
                        nc.vector.tensor_tensor(out=h4[:], in0=h4[:], in1=g4[:], op=MUL)
                        nc.vector.tensor_tensor(out=Y3, in0=h4[:], in1=X3, op=ADD)
                        if X8out is not None:
                            x8_copy(Y3, X8out, lo, hi)
                    elif X8out is not None:
                        x8_copy(c3(Y, lo, hi), X8out, lo, hi)
                    if post_chunk is not None:
                        post_chunk(n)

            with scope("hw0l0"):
                x1b = act_buf("actB")
                x8b = x8_buf()
                highway_layer(x1, x1b, 0, x8a, x8b)

            # hw0l1 with interleaved conv1-mask application: zero the masked
            # columns (conv1 left padding), then an fp8 copy of the masked
            # x1c (with 2-col halo) feeds conv1's DoubleRow matmuls
            with scope("hw0l1"):
                x1c = act_buf("actA")
                x8c = x8_buf()
                nc.vector.memset(x8c[:, :, 0:2], 0)
                nc.vector.memset(x8c[:, :, 2 + W:WP8], 0)

                def m1c_chunk(n):
                    lo, hi = n * w, (n + 1) * w
                    for q in range(4):
                        s = x1c[:, q * WB + HALO + lo:q * WB + HALO + hi]
                        nc.vector.tensor_tensor(out=s, in0=s,
                                                in1=t_m1[:, lo:hi], op=MUL)
                    nc.vector.tensor_scalar(out=x8c[:, :, 2 + lo:2 + hi],
                                            in0=c3(x1c, lo, hi),
                                            scalar1=SX, scalar2=None, op0=MUL)

                def post_m1c(n):
                    if n >= 1:
                        m1c_chunk(n - 1)

                highway_layer(x1b, x1c, 1, x8b, None, post_chunk=post_m1c)
                m1c_chunk(nch - 1)

            # ---------- conv1 (fp8 DR, +residual) + fp8 copy for hw1l0 ----
            with scope("conv1"):
                x2p = act_buf("actB")
                x8a2 = x8_buf()
                for n in range(nch):
                    lo, hi = n * w, (n + 1) * w
                    r4 = tp.tile([128, 4 * w], F16, tag="r", name="r4")
                    for m in range(4):
                        ps = pp.tile([128, w], F32, tag="ps", name="ps")
                        i = 0
                        for k in range(3):
                            for p in range(2):
                                idx = ((m * 3 + k) * 2 + p) * 2
                                nc.tensor.matmul(
                                    out=ps[:], lhsT=t_wc18[:, idx:idx + 2, :],
                                    rhs=x8c[:, 2 * p:2 * p + 2,
                                            lo + k + 1:hi + k + 1],
                                    start=(i == 0), stop=(i == 5), perf_mode=DR)
                                i += 1
                        nc.scalar.activation(out=r4[:, m * w:(m + 1) * w],
                                             in_=ps[:], func=RELU,
                                             bias=t_bc1[:, m:m + 1], scale=SINV)
                    Y3 = c3(x2p, lo, hi)
                    nc.vector.tensor_tensor(out=Y3, in0=r4[:], in1=c3(x1c, lo, hi),
                                            op=ADD)
                    x8_copy(Y3, x8a2, lo, hi)

            # ---------- hw1l0 + hw1l1 (in-place on x2b) + pool/proj, fully
            # chunk-pipelined so the downstream ACT/DVE chains spread across
            # the whole window ----------
            with scope("hw1l1"):
                x2b = act_buf("actA")
                x8b2 = x8_buf()
                x2 = x2b

                def poolproj_chunk(n):
                    lo, hi = n * w, (n + 1) * w
                    mq = tp.tile([128, 4 * w], F16, tag="mq", name="mq", bufs=2)
                    for c in range(4):
                        base = c * WB + HALO
                        s1 = tp.tile([128, w], F16, tag="s1", name="s1", bufs=2)
                        s2 = tp.tile([128, w], F16, tag="s2", name="s2", bufs=1)
                        nc.vector.tensor_tensor(out=s1[:], in0=x2[:, base + 1 + lo:base + 1 + hi],
                                                in1=t_am[:, lo:hi], op=ADD)
                        nc.vector.tensor_tensor(out=s2[:], in0=x2[:, base + 2 + lo:base + 2 + hi],
                                                in1=t_am[:, W + lo:W + hi], op=ADD)
                        nc.vector.tensor_tensor(out=s1[:], in0=s1[:], in1=s2[:], op=MAX)
                        nc.vector.tensor_tensor(out=mq[:, c * w:(c + 1) * w], in0=s1[:],
                                                in1=x2[:, base + lo:base + hi], op=MAX)
                    for m in range(4):
                        ps = pp.tile([128, w], F32, tag="ps", name="ps")
                        for q in range(4):
                            nc.tensor.matmul(
                                out=ps[:],
                                lhsT=t_wpr[:, q * WED + m * 128:q * WED + (m + 1) * 128],
                                rhs=mq[:, q * w:(q + 1) * w],
                                start=(q == 0), stop=(q == 3))
                        o_t = tp.tile([128, w], F16, tag="o", name="o_t", bufs=4)
                        nc.scalar.activation(out=o_t[:], in_=ps[:], func=IDEN,
                                             bias=t_bpr[:, m:m + 1], scale=1.0)
                        nc.sync.dma_start(out=out[m * 128:m * 128 + 64, lo:hi],
                                          in_=o_t[0:64, :])
                        nc.scalar.dma_start(out=out[m * 128 + 64:(m + 1) * 128, lo:hi],
                                            in_=o_t[64:128, :])

                for n in range(nch):
                    highway_layer(x2p, x2b, 2, x8a2, x8b2, chunks=[n])
                    if n >= 1:
                        highway_layer(x2b, x2b, 3, x8b2, None, fused=False,
                                      chunks=[n - 1])
                    if n >= 3:
                        poolproj_chunk(n - 3)
                highway_layer(x2b, x2b, 3, x8b2, None, fused=False,
                              chunks=[nch - 1])
                for k in (nch - 3, nch - 2, nch - 1):
                    poolproj_chunk(k)

    nc.compile()
    return nc


# ---------------------------------------------------------------- host prep

def _prep_inputs(inputs):
    pl = np.asarray(inputs["pool_lengths"], np.int64)
    toks = np.asarray(inputs["byte_tokens"], np.int64)
    bpe = np.asarray(inputs["bpe_mask"], bool)
    emb = np.asarray(inputs["tok_emb"], np.float32)
    starts = np.cumsum(pl, axis=1) - pl

    streams, zcols, cap = _plan_packing(pl)
    nch, w = _chunk_plan(cap)
    W = nch * w
    WB = W + 8

    def f16(x):
        return np.ascontiguousarray(np.asarray(x, np.float32).astype(_F16_NP))

    conv0_W = np.asarray(inputs["conv0_W"], np.float32)   # [3,128,512]
    conv1_W = np.asarray(inputs["conv1_W"], np.float32)   # [3,512,512]
    hw0_W = np.asarray(inputs["hw0_W"], np.float32)       # [2,1024,512]
    hw1_W = np.asarray(inputs["hw1_W"], np.float32)
    proj_W = np.asarray(inputs["proj_W"], np.float32)     # [512,512]

    w_c0 = f16(conv0_W.transpose(1, 0, 2).reshape(128, 3 * WED))
    w_c18 = np.empty((128, 48, 128), np.float32)
    for m in range(4):
        for k in range(3):
            for p in range(2):
                for j in range(2):
                    idx = ((m * 3 + k) * 2 + p) * 2 + j
                    w_c18[:, idx, :] = SW * conv1_W[k, (2 * p + j) * 128:
                                                    (2 * p + j + 1) * 128,
                                                    m * 128:(m + 1) * 128]
    w_c18 = np.ascontiguousarray(w_c18.astype(_F8_NP))
    layers = ((hw0_W, 0), (hw0_W, 1), (hw1_W, 0), (hw1_W, 1))
    whh = np.empty((128, 16, WED), np.float32)
    wg8 = np.empty((128, 64, 128), np.float32)
    for bl, (blk, lay) in enumerate(layers):
        wt = blk[lay]                       # [1024, 512]; rows: h 0:512, g 512:1024
        for q in range(4):
            whh[:, bl * 4 + q, :] = wt[:WED, q * 128:(q + 1) * 128].T
        for m in range(4):
            for p in range(2):
                for j in range(2):
                    idx = ((bl * 4 + m) * 2 + p) * 2 + j
                    # lhsT[ki, m_col] = Wg[out=(m*128+m_col), in=(2p+j)*128+ki]
                    wg8[:, idx, :] = SW * wt[WED + m * 128:WED + (m + 1) * 128,
                                             (2 * p + j) * 128:(2 * p + j + 1) * 128].T
    w_hh = f16(whh.reshape(128, 16 * WED))
    w_g8 = np.ascontiguousarray(wg8.astype(_F8_NP))
    w_pr = f16(proj_W.T.reshape(4, 128, WED).transpose(1, 0, 2).reshape(128, 4 * WED))

    def colchunks(b):
        return np.ascontiguousarray(np.asarray(b, np.float32).reshape(4, 128).T)

    b_c0 = colchunks(inputs["conv0_b"])
    b_c1 = colchunks(inputs["conv1_b"])
    bhw = np.empty((128, 4, 8), np.float32)
    for bl, (blk, lay) in enumerate((("hw0_b", 0), ("hw0_b", 1), ("hw1_b", 0), ("hw1_b", 1))):
        b = np.asarray(inputs[blk], np.float32)[lay]
        bhw[:, bl, 0:4] = b[:512].reshape(4, 128).T
        bhw[:, bl, 4:8] = b[512:1024].reshape(4, 128).T
    b_hw = np.ascontiguousarray(bhw.reshape(128, 32))
    b_pr = colchunks(inputs["proj_b"])

    shared = dict(w_c0=w_c0, w_c18=w_c18, w_hh=w_hh, w_g8=w_g8, w_pr=w_pr,
                  b_c0=b_c0, b_c1=b_c1, b_hw=b_hw, b_pr=b_pr)

    in_maps = []
    gathers = []
    for core in range(N_CORES):
        m = dict(shared)
        x0 = np.zeros((128, WB), np.float32)
        a12 = np.full((2, W), NEG, np.float32)
        msk = np.ones(W, np.float32)
        gb, gw, gc = [], [], []
        for fr in streams[core]:
            b, s_lo, s_hi, pos = fr["b"], fr["s_lo"], fr["s_hi"], fr["pos"]
            fl = s_hi - s_lo
            tt = toks[b, s_lo:s_hi]
            x = emb[tt] + np.where(bpe[b, s_lo:s_hi, None], emb[BPE_MASK_IDX][None, :], 0.0)
            x0[:, HALO + pos:HALO + pos + fl] = x.T
            plw = pl[b, fr["w0"]:fr["w1"]]
            st = starts[b, fr["w0"]:fr["w1"]]
            cols = pos + (st - s_lo)
            a12[0, cols[plw > 1]] = 0.0
            a12[1, cols[plw > 2]] = 0.0
            nz = plw > 0
            gb.extend([b] * int(nz.sum()))
            gw.extend(np.arange(fr["w0"], fr["w1"])[nz].tolist())
            gc.extend(cols[nz].tolist())
        for z in zcols[core]:
            msk[z] = 0.0
        m["x0_in"] = x0.astype(_F16_NP)
        m["a_msk"] = np.ascontiguousarray(
            np.broadcast_to(a12.reshape(1, 2 * W), (128, 2 * W)).astype(_F16_NP))
        m["m1c"] = np.ascontiguousarray(
            np.broadcast_to(msk[None, :], (128, W)).astype(_F8_NP))
        in_maps.append(m)
        gathers.append((np.asarray(gb), np.asarray(gw), np.asarray(gc)))
    meta = dict(gathers=gathers, nch=nch, w=w)
    return in_maps, meta


def kernel(**inputs) -> np.ndarray:
    from concourse.bass_utils import run_bass_kernel_spmd

    in_maps, meta = _prep_inputs(inputs)
    key = (meta["nch"], meta["w"])
    if _CACHE.get("key") != key:
        _CACHE["nc"] = _build_program(*key)
        _CACHE["key"] = key
    nc = _CACHE["nc"]

    res = run_bass_kernel_spmd(nc, in_maps, list(range(N_CORES)))

    proj_b = np.asarray(inputs["proj_b"], np.float32)
    full = np.empty((BSZ, NW, WED), np.float32)
    full[:] = proj_b[None, None, :]
    for core in range(N_CORES):
        o = np.asarray(res.results[core]["out"], np.float32)  # [512, W]
        gb, gw, gc = meta["gathers"][core]
        if len(gb):
            full[gb, gw] = o[:, gc].T
    return full
